# revision 1
# baseline (speedup 1.0000x reference)
"""RBF-kernel causal attention on 8 Trainium2 NeuronCores.

B=2, H=16, N=2048, D=64. Shards the 32 (b,h) attention instances across 8
cores (4 heads per core). Math notes:

  logits = -relu(||q-k||^2)/sqrt(D); relu is a no-op (||q-k||^2 >= 0 up to
  rounding), and softmax is invariant to per-query offsets, so
      softmax_n(-(qsq_m + ksq_n - 2 qk)/8) == softmax_n(qk/4 - ksq_n/8)
  We compute P'' = exp(0.25 * K Q^T) in a [key, query] layout and fold the
  exp(-0.125 ksq_n) per-key factor into V (and into the appended ones-column
  that produces the softmax denominator):
      [O^T | l] accumulates via matmul(lhsT=V_aug_scaled, rhs=P'').
  Final output O[m,d] = OT[d,m] / l[m], un-transposed via PE transpose.

Emission is manually software-pipelined: head h+1's setup chunks (transposes,
ksq, V scaling) are interleaved between head h's query blocks so the tile
scheduler (limited lookahead) can overlap them.
"""

import sys

if "/opt/trn_rl_repo" not in sys.path:
    sys.path.insert(0, "/opt/trn_rl_repo")

import numpy as np

import concourse.bacc as bacc
import concourse.mybir as mybir
import concourse.tile as tile
from concourse.masks import make_identity

B, H, N, D = 2, 16, 2048, 64
NCORES = 8
HPC = (B * H) // NCORES  # heads per core = 4
P = 128                  # partitions
NT = N // P              # key tiles per head = 16
QB = 512                 # query block (matmul moving dim)
MBS = N // QB            # query blocks per head = 4
G = 2                    # key tiles per exp/ACT group (2 PSUM banks)

F32 = mybir.dt.float32
# float32r = relaxed-precision fp32 matmul (1 cycle/row at moving dim >= 256
# instead of 4 for float32)
MM_DT = mybir.dt.float32r


def build_nc():
    nc = bacc.Bacc("TRN2", target_bir_lowering=False, debug=False)
    q = nc.dram_tensor("q", [HPC, N, D], F32, kind="ExternalInput")
    k = nc.dram_tensor("k", [HPC, N, D], F32, kind="ExternalInput")
    v = nc.dram_tensor("v", [HPC, N, D], F32, kind="ExternalInput")
    out = nc.dram_tensor("out", [HPC, N, D], F32, kind="ExternalOutput")

    with tile.TileContext(nc) as tc:
        with (
            tc.tile_pool(name="const", bufs=1) as const_pool,
            tc.tile_pool(name="loads", bufs=1) as load_pool,
            tc.tile_pool(name="head", bufs=2) as head_pool,
            tc.tile_pool(name="work", bufs=3) as work_pool,
            tc.tile_pool(name="p", bufs=4) as p_pool,
            tc.tile_pool(name="epi", bufs=3) as epi_pool,
            tc.tile_pool(name="st", bufs=3, space="PSUM") as st_pool,
            tc.tile_pool(name="otp", bufs=2, space="PSUM") as ot_pool,
        ):
            identity = const_pool.tile([P, P], F32)
            make_identity(nc, identity)
            # multiplicative causal masks for odd diagonal tiles (jj=1,3):
            # mask[jj][n, m] = 1.0 if m - n - 128*jj >= 0 else 0.0
            dmask = {}
            for jj in (1, 3):
                mk = const_pool.tile([P, QB], F32, tag=f"dmask{jj}", name="mk")
                nc.gpsimd.memset(mk[:], 1.0)
                nc.gpsimd.affine_select(
                    out=mk[:], in_=mk[:],
                    compare_op=mybir.AluOpType.is_ge, fill=0.0,
                    base=-P * jj, pattern=[[1, QB]], channel_multiplier=-1,
                )
                dmask[jj] = mk

            # prefetch every head's inputs up front: no-wait DMAs stream in
            # the background while compute proceeds
            knats, qnats, vtmps = [], [], []
            for h in range(HPC):
                # quarter-granular loads so the first transposes start as
                # soon as the first chunk lands, not after the whole head
                knat = load_pool.tile([P, NT, D], F32, tag=f"knat{h}")
                # q loaded DOUBLED along a repeat dim (two passes over DRAM):
                # transposing [128m, (2,64d)] then yields Q^T duplicated on
                # both partition halves, as the row-packed matmuls need
                qnat = load_pool.tile([P, NT, 2, D], F32, tag=f"qnat{h}")
                vtmp = load_pool.tile([P, NT, D], F32, tag=f"vtmp{h}")
                kq = k[h].rearrange("(t p) d -> p t d", p=P)
                qq = q[h].rearrange("(t p) d -> p t d", p=P)
                vq = v[h].rearrange("(t p) d -> p t d", p=P)
                nch = 4 if h == 0 else 1
                w_ = NT // nch
                for c in range(nch):
                    ts = slice(w_ * c, w_ * c + w_)
                    nc.sync.dma_start(knat[:, ts, :], kq[:, ts, :])
                    for r in range(2):
                        nc.sync.dma_start(qnat[:, ts, r, :], qq[:, ts, :])
                    nc.sync.dma_start(vtmp[:, ts, :], vq[:, ts, :])
                knats.append(knat)
                qnats.append(qnat)
                vtmps.append(vtmp)

            heads = [{} for _ in range(HPC)]

            def setup_chunks(h):
                """Emission chunks for head h's setup, in dependency order."""
                st = heads[h]

                def allocs():
                    st["ksq"] = head_pool.tile([P, NT], F32, tag="ksq", name="ksq")
                    st["w"] = head_pool.tile([P, NT], F32, tag="w", name="w")
                    st["vaug"] = head_pool.tile(
                        [P, NT, D + 1], MM_DT, tag="vaug", name="vaug"
                    )
                    # kt: key-tile PAIRS stacked on partition halves
                    # (even tile at partitions 0:64, odd at 64:128) so two
                    # QK matmuls can row-pack the PE array concurrently.
                    st["kt"] = head_pool.tile(
                        [P, NT // 2, P], MM_DT, tag="kt", name="kt"
                    )
                    # qt: Q^T duplicated into both partition halves (the
                    # row-packed matmuls stream rhs partitions 0:64 and
                    # 64:128 into array row groups 0-1 and 2-3)
                    st["qt"] = head_pool.tile([P, NT, P], MM_DT, tag="qt", name="qt")

                def scale_chunk(c, nt4=4):
                    # per-quarter V_aug build: runs as soon as that quarter
                    # of k and v has landed (head 0 only; later heads build
                    # whole-head to save per-instruction overhead)
                    def run():
                        ts = slice(4 * c, 4 * c + nt4)
                        knat, vtmp = knats[h], vtmps[h]
                        ksq, w, vaug = st["ksq"], st["w"], st["vaug"]
                        ktmp = work_pool.tile([P, nt4, D], F32, tag="ktmp")
                        nc.vector.tensor_mul(
                            out=ktmp[:], in0=knat[:, ts, :], in1=knat[:, ts, :]
                        )
                        nc.vector.tensor_reduce(
                            ksq[:, ts], ktmp[:],
                            axis=mybir.AxisListType.X, op=mybir.AluOpType.add,
                        )
                        nc.scalar.activation(
                            w[:, ts], ksq[:, ts],
                            mybir.ActivationFunctionType.Exp, scale=-0.125,
                        )
                        nc.gpsimd.tensor_mul(
                            out=vaug[:, ts, :D],
                            in0=vtmp[:, ts, :],
                            in1=w[:, ts, None].to_broadcast((P, nt4, D)),
                        )
                        nc.gpsimd.tensor_copy(
                            out=vaug[:, ts, D : D + 1], in_=w[:, ts, None]
                        )

                    return run

                def ktr_group(g):
                    # 4 pair-transposes: [128n, (2t, 64d)] -> [(2t, 64d), 128n]
                    # lands even tile at partitions 0:64, odd at 64:128
                    def run():
                        src = knats[h]
                        dst = heads[h]["kt"]
                        tp = st_pool.tile([P, 4, P], F32, tag="stg", name="tp")
                        for j in range(4):
                            pr = 4 * g + j
                            nc.tensor.transpose(
                                tp[:, j, :], src[:, 2 * pr : 2 * pr + 2, :],
                                identity[:],
                            )
                        nc.vector.tensor_copy(
                            out=dst[:, 4 * g : 4 * g + 4, :], in_=tp[:]
                        )

                    return run

                def qtr_group(g):
                    # transpose a 0-stride doubled view [128m, (2, 64d)] so
                    # the output holds Q^T duplicated on both partition
                    # halves (rows 0:64 and 64:128) in one shot
                    def run():
                        src = qnats[h]
                        dst = heads[h]["qt"]
                        tp = st_pool.tile([P, 4, P], F32, tag="stg", name="tp")
                        for j in range(4):
                            nc.tensor.transpose(
                                tp[:, j, :], src[:, 4 * g + j, :, :], identity[:]
                            )
                        nc.vector.tensor_copy(
                            out=dst[:, 4 * g : 4 * g + 4, :], in_=tp[:]
                        )

                    return run

                # query block mb needs kt pair-groups up to (2mb+1)//4, qt
                # group mb, and vaug quarter mb; yield in dependency order
                yield allocs
                if h == 0:
                    yield scale_chunk(0)
                    yield ktr_group(0)
                    yield qtr_group(0)
                    yield scale_chunk(1)
                    yield qtr_group(1)
                    yield ktr_group(1)
                    yield scale_chunk(2)
                    yield qtr_group(2)
                    yield scale_chunk(3)
                    yield qtr_group(3)
                else:
                    yield scale_chunk(0, NT)
                    yield ktr_group(0)
                    yield qtr_group(0)
                    yield qtr_group(1)
                    yield ktr_group(1)
                    yield qtr_group(2)
                    yield qtr_group(3)

            def job_chunks(h, mb):
                """Chunks of one (head, query-block) job, for interleaving."""
                kt, qt, vaug = heads[h]["kt"], heads[h]["qt"], heads[h]["vaug"]
                nsub = 4 * mb          # sub-diagonal key tiles
                qt_lo = qt[:D, 4 * mb : 4 * mb + 4, :]   # [64, 512]
                qt_hi = qt[D:, 4 * mb : 4 * mb + 4, :]   # [64, 512]
                ntiles = nsub + 4
                jst = {"prev": None, "ot": None}

                def sub_group(s):
                    def run():
                        if jst["ot"] is None:
                            jst["ot"] = ot_pool.tile(
                                [D + 1, QB], F32, tag="ot", name="ot"
                            )
                        stg = st_pool.tile([P, G, QB], F32, tag="stg")
                        pr = s // 2
                        nc.tensor.matmul(
                            stg[:, 0, :], kt[:D, pr, :], qt_lo,
                            start=True, stop=True, skip_group_check=True,
                        )
                        nc.tensor.matmul(
                            stg[:, 1, :], kt[D:, pr, :], qt_hi,
                            start=True, stop=True, skip_group_check=True,
                        )
                        pg = p_pool.tile([P, G, QB], MM_DT, tag="pg")
                        nc.scalar.activation(
                            pg[:], stg[:],
                            mybir.ActivationFunctionType.Exp, scale=0.25,
                        )
                        if jst["prev"] is not None:
                            _emit_pv(nc, jst["ot"], vaug, jst["prev"], ntiles)
                        jst["prev"] = (pg, [s, s + 1])

                    return run

                def diag_group(a):
                    def run():
                        if jst["ot"] is None:
                            jst["ot"] = ot_pool.tile(
                                [D + 1, QB], F32, tag="ot", name="ot"
                            )
                        if a == 0:
                            jst["pgd"] = p_pool.tile([P, 4, QB], MM_DT, tag="pgd", name="pgd")
                        pgd = jst["pgd"]
                        # columns m < 128*(2a) of tiles (2a, 2a+1) are fully
                        # masked: skip their QK matmul + exp; affine_select
                        # below zero-fills that (otherwise garbage) region.
                        c0 = P * 2 * a
                        stg = st_pool.tile([P, G, QB], F32, tag="stg")
                        pr = 2 * mb + a
                        nc.tensor.matmul(
                            stg[:, 0, c0:],
                            kt[:D, pr, :],
                            qt[:D, 4 * mb + 2 * a : 4 * mb + 4, :],
                            start=True, stop=True, skip_group_check=True,
                        )
                        nc.tensor.matmul(
                            stg[:, 1, c0:],
                            kt[D:, pr, :],
                            qt[D:, 4 * mb + 2 * a : 4 * mb + 4, :],
                            start=True, stop=True, skip_group_check=True,
                        )
                        nc.scalar.activation(
                            pgd[:, 2 * a : 2 * a + 2, c0:], stg[:, :, c0:],
                            mybir.ActivationFunctionType.Exp, scale=0.25,
                        )
                        # keep pgd[n, jj, m] iff m - n - 128 jj >= 0; the
                        # even tile masks on Pool (affine_select), the odd
                        # ones concurrently on DVE (mul by a const mask)
                        jj = 2 * a
                        nc.gpsimd.affine_select(
                            out=pgd[:, jj, :], in_=pgd[:, jj, :],
                            compare_op=mybir.AluOpType.is_ge, fill=0.0,
                            base=-P * jj, pattern=[[1, QB]],
                            channel_multiplier=-1,
                        )
                        nc.vector.tensor_mul(
                            out=pgd[:, jj + 1, :],
                            in0=pgd[:, jj + 1, :],
                            in1=dmask[jj + 1][:],
                        )

                    return run

                def pv_epilogue():
                    ot, pgd = jst["ot"], jst["pgd"]
                    if jst["prev"] is not None:
                        _emit_pv(nc, ot, vaug, jst["prev"], ntiles)
                    for j in range(4):
                        nc.tensor.matmul(
                            ot[:], vaug[:, 4 * mb + j, :], pgd[:, j, :],
                            start=(nsub == 0 and j == 0), stop=(j == 3),
                            skip_group_check=True,
                        )
                    # epilogue: transpose + normalize + store
                    ot_sb = epi_pool.tile([D + 1, QB], F32, tag="ot_sb")
                    nc.vector.tensor_copy(out=ot_sb[:], in_=ot[:])
                    tpo = ot_pool.tile([P, 4, D + 1], F32, tag="ot", name="tpo")
                    for j in range(4):
                        nc.tensor.transpose(
                            tpo[:, j, :],
                            ot_sb[:, j * P : (j + 1) * P],
                            identity[: D + 1, : D + 1],
                        )
                    linv = epi_pool.tile([P, 4], F32, tag="linv")
                    nc.vector.reciprocal(linv[:], tpo[:, :, D])
                    o_sb = epi_pool.tile([P, 4, D], F32, tag="o_sb")
                    for j in range(4):
                        nc.vector.tensor_scalar_mul(
                            o_sb[:, j, :], tpo[:, j, :D], linv[:, j : j + 1]
                        )
                    nc.sync.dma_start(
                        out[h, mb * QB : (mb + 1) * QB, :].rearrange(
                            "(j p) d -> p j d", p=P
                        ),
                        o_sb[:],
                    )

                chunks = [sub_group(s) for s in range(0, nsub, G)]
                chunks += [diag_group(0), diag_group(1), pv_epilogue]
                return chunks

            # ---- software-pipelined emission: depth-2 job interleave ----
            for c in setup_chunks(0):
                c()
            pending = []          # next head's setup chunks, dripped in
            jobs = [(h, mb) for h in range(HPC) for mb in range(MBS)]
            active = []           # up to 2 jobs' chunk queues
            ji = 0
            drip = 0
            while active or ji < len(jobs):
                while len(active) < 2 and ji < len(jobs):
                    h, mb = jobs[ji]
                    if mb == 0 and pending:
                        # head h's setup must be fully emitted before its
                        # first job
                        for c in pending:
                            c()
                        pending = []
                    if mb == 0 and h + 1 < HPC:
                        pending = list(setup_chunks(h + 1))
                    active.append(job_chunks(h, mb))
                    ji += 1
                for q_ in list(active):
                    q_.pop(0)()
                    drip += 1
                    if drip % 3 == 0 and pending:
                        pending.pop(0)()
                active = [q_ for q_ in active if q_]
            for c in pending:
                c()

    nc.compile()
    return nc


def _emit_pv(nc, ot, vaug, group, ntiles):
    pg, tiles = group
    for j, nt in enumerate(tiles):
        nc.tensor.matmul(
            ot[:],
            vaug[:, nt, :],
            pg[:, j, :],
            start=(nt == 0),
            stop=(nt == ntiles - 1),
            skip_group_check=True,
        )


_NC = None


def _get_nc():
    global _NC
    if _NC is None:
        _NC = build_nc()
    return _NC


def kernel(q: np.ndarray, k: np.ndarray, v: np.ndarray) -> np.ndarray:
    from concourse.bass_utils import run_bass_kernel_spmd

    nc = _get_nc()
    qf = np.ascontiguousarray(np.asarray(q, dtype=np.float32).reshape(B * H, N, D))
    kf = np.ascontiguousarray(np.asarray(k, dtype=np.float32).reshape(B * H, N, D))
    vf = np.ascontiguousarray(np.asarray(v, dtype=np.float32).reshape(B * H, N, D))
    in_maps = [
        {
            "q": np.ascontiguousarray(qf[c * HPC : (c + 1) * HPC]),
            "k": np.ascontiguousarray(kf[c * HPC : (c + 1) * HPC]),
            "v": np.ascontiguousarray(vf[c * HPC : (c + 1) * HPC]),
        }
        for c in range(NCORES)
    ]
    res = run_bass_kernel_spmd(nc, in_maps, core_ids=list(range(NCORES)))
    outs = [res.results[c]["out"] for c in range(NCORES)]
    return np.concatenate(outs, axis=0).reshape(B, H, N, D)


if __name__ == "__main__":
    rng = np.random.default_rng(0)
    qq = rng.standard_normal((B, H, N, D), dtype=np.float32)
    kk = rng.standard_normal((B, H, N, D), dtype=np.float32)
    vv = rng.standard_normal((B, H, N, D), dtype=np.float32)
    o = kernel(q=qq, k=kk, v=vv)
    print("kernel ran, out shape", o.shape, "finite:", np.isfinite(o).all())



# revision 22
# speedup vs baseline: 1.1692x; 1.1692x over previous
"""RBF-kernel causal attention on 8 Trainium2 NeuronCores.

B=2, H=16, N=2048, D=64. Shards the 32 (b,h) attention instances across 8
cores (4 heads per core). Math notes:

  logits = -relu(||q-k||^2)/sqrt(D); relu is a no-op (||q-k||^2 >= 0 up to
  rounding), and softmax is invariant to per-query offsets, so
      softmax_n(-(qsq_m + ksq_n - 2 qk)/8) == softmax_n(qk/4 - ksq_n/8)
  We compute pg = exp(0.25 * K Q^T) in a [key, query] layout (bf16) and fold
  the exp(-0.125 ksq_n) per-key factor into V (and into the appended
  ones-column that produces the softmax denominator):
      O_aug[q, 0:65] accumulates via matmul(lhsT=pg_slice, rhs=V_aug_scaled)
  directly in the natural [query, feature] layout, so no output transpose is
  needed. Final O[q, d] = O_aug[q, d] / O_aug[q, 64].

Engine plan (per core): ACT does all the exps (bottleneck ~75us); PE does QK
(row-packed bf16 halves) + PV; DMA engines do the K^T/Q^T transposes
(batched dma_start_transpose, 14ns per 16x128 xbar tile); Pool does the
f32->bf16 converts; DVE does ksq, causal-mask muls (bf16 2x mode), and the
epilogue divide.

Keys and queries are pair-interleaved in SBUF (partition p of a 256-row
chunk holds rows {2p, 2p+1}) so every DMA descriptor moves 512B contiguous
(2x fewer descriptors); softmax is permutation-invariant over keys, and the
causal masks / store patterns account for the query permutation.
"""

import sys

if "/opt/trn_rl_repo" not in sys.path:
    sys.path.insert(0, "/opt/trn_rl_repo")

import numpy as np

import concourse.bacc as bacc
import concourse.mybir as mybir
import concourse.tile as tile

B, H, N, D = 2, 16, 2048, 64
NCORES = 8
HPC = (B * H) // NCORES  # heads per core = 4
P = 128                  # partitions
CH = N // 256            # 256-key chunks per head = 8
QB = 512                 # query block
MBS = N // QB            # query blocks per head = 4
NT = 2 * CH              # 128-key tiles per head = 16 (chunk t, parity r)

F32 = mybir.dt.float32
BF16 = mybir.dt.bfloat16
EXP = mybir.ActivationFunctionType.Exp


def build_nc():
    nc = bacc.Bacc("TRN2", target_bir_lowering=False, debug=False)
    q = nc.dram_tensor("q", [HPC, N, D], F32, kind="ExternalInput")
    k = nc.dram_tensor("k", [HPC, N, D], F32, kind="ExternalInput")
    v = nc.dram_tensor("v", [HPC, N, D], F32, kind="ExternalInput")
    out = nc.dram_tensor("out", [HPC, N, D], F32, kind="ExternalOutput")

    with tile.TileContext(nc) as tc:
        with (
            tc.tile_pool(name="const", bufs=1) as const_pool,
            tc.tile_pool(name="loads", bufs=1) as load_pool,
            tc.tile_pool(name="head", bufs=2) as head_pool,
            tc.tile_pool(name="pg", bufs=20) as pg_pool,
            tc.tile_pool(name="epi", bufs=3) as epi_pool,
            tc.tile_pool(name="stg", bufs=3, space="PSUM") as stg_pool,
            tc.tile_pool(name="ob", bufs=2, space="PSUM") as ob_pool,
        ):
            # causal mask for the diagonal 256-key chunk vs its own 256
            # queries, in the pair-interleaved (key = 2p + r, query =
            # 2p' + r') coordinates: M[p, r, r'*128 + p'] = 1 iff
            # 2p' + r' >= 2p + r.
            M = const_pool.tile([P, 2, 256], BF16, name="mask")
            nc.gpsimd.memset(M[:], 1.0)
            for r in range(2):
                for rp in range(2):
                    nc.gpsimd.affine_select(
                        out=M[:, r, 128 * rp : 128 * rp + 128],
                        in_=M[:, r, 128 * rp : 128 * rp + 128],
                        compare_op=mybir.AluOpType.is_ge, fill=0.0,
                        base=rp - r, pattern=[[2, P]], channel_multiplier=-2,
                    )


            # per-head input tiles; the DMA loads are emitted inside
            # setup_chunks so each head's transposes don't queue behind
            # later heads' loads on SP's in-order sequencer.
            # Pair-interleaved: nat[p, t, r, d] = x[256t + 2p + r, d] so each
            # descriptor is 512B ((r, d) contiguous in DRAM).
            knats, qnats, vnats = [], [], []
            for h in range(HPC):
                knats.append(load_pool.tile([P, CH, 2, D], F32, tag=f"knat{h}", name="kn"))
                qnats.append(load_pool.tile([P, CH, 2, D], F32, tag=f"qnat{h}", name="qn"))
                vnats.append(load_pool.tile([P, CH, 2, D], F32, tag=f"vnat{h}", name="vn"))

            heads = [{} for _ in range(HPC)]

            def setup_chunks(h):
                """Emission chunks for head h's setup, in dependency order."""
                st = heads[h]

                def allocs():
                    # everything is allocated per 2-chunk piece: single-writer
                    # tiles keep the scheduler's RAW waits precise (a reader
                    # of a multi-writer tile waits for ALL its writers)
                    npc = CH // 2
                    st["kbf"] = [
                        head_pool.tile([P, 2, 2, D], BF16, tag=f"kbf{pi}", name="kbf")
                        for pi in range(npc)
                    ]
                    st["qbfd"] = [
                        head_pool.tile([P, 2, 2, 2, D], BF16, tag=f"qbfd{pi}", name="qbfd")
                        for pi in range(npc)
                    ]
                    # kt[:, t, :]: partitions r*64+d hold K^T of parity-r keys
                    # of chunk t; free j = key (256t + 2j + r)
                    st["kt"] = [
                        head_pool.tile([P, 2, P], BF16, tag=f"kt{pi}", name="kt")
                        for pi in range(npc)
                    ]
                    # qt[:, i, :]: Q^T of query tile i=(t', r'), duplicated on
                    # both partition halves (for the row-packed QK matmuls)
                    st["qt"] = [
                        head_pool.tile([P, 4, P], BF16, tag=f"qt{pi}", name="qt")
                        for pi in range(npc)
                    ]
                    st["ksq"] = [
                        head_pool.tile([P, 4], F32, tag=f"ksq{pi}", name="ksq")
                        for pi in range(npc)
                    ]
                    st["w"] = [
                        head_pool.tile([P, 4], F32, tag=f"w{pi}", name="w")
                        for pi in range(npc)
                    ]
                    st["vaug"] = [
                        head_pool.tile([P, 4, D + 1], BF16, tag=f"vaug{pi}", name="vaug")
                        for pi in range(npc)
                    ]

                def loads(c0, cw):
                    eng = nc.sync
                    def run():
                        cs = slice(c0, c0 + cw)
                        eng.dma_start(
                            knats[h][:, cs],
                            k[h].rearrange("(t p r) d -> p t r d", p=P, r=2)[:, cs],
                        )
                        eng.dma_start(
                            qnats[h][:, cs],
                            q[h].rearrange("(t p r) d -> p t r d", p=P, r=2)[:, cs],
                        )
                        eng.dma_start(
                            vnats[h][:, cs],
                            v[h].rearrange("(t p r) d -> p t r d", p=P, r=2)[:, cs],
                        )

                    return run

                def piece(pi):
                    # one 2-chunk piece of the convert + transpose pipeline
                    def run():
                        cs = slice(2 * pi, 2 * pi + 2)
                        kbf, qbfd = st["kbf"][pi], st["qbfd"][pi]
                        knat, qnat = knats[h], qnats[h]
                        nc.gpsimd.tensor_copy(out=kbf[:], in_=knat[:, cs])
                        nc.gpsimd.tensor_copy(
                            out=qbfd[:],
                            in_=qnat[:, cs, :, None, :].to_broadcast(
                                (P, 2, 2, 2, D)
                            ),
                        )
                        nc.sync.dma_start_transpose(
                            st["kt"][pi][:],
                            kbf[:].rearrange("p a b c -> p (a b c)"),
                        )
                        nc.sync.dma_start_transpose(
                            st["qt"][pi][:],
                            qbfd[:].rearrange("p a b c d -> p (a b c d)"),
                        )

                    return run

                def waug(pi):
                    # ksq -> w -> vaug for the piece's 4 key tiles. Emitted
                    # AFTER the head's transfer pieces (dripped into the job
                    # stream): the w-exp would otherwise sit at the head of
                    # ACT's in-order queue waiting on the DVE ksq chain and
                    # gate all the main exps behind it.
                    def run():
                        cs = slice(2 * pi, 2 * pi + 2)
                        knat, vnat = knats[h], vnats[h]
                        ksq, w, vaug_ = st["ksq"][pi], st["w"][pi], st["vaug"][pi]
                        ktmp = epi_pool.tile([P, 2, 2, D], F32, tag="ktmp")
                        nc.vector.tensor_mul(
                            out=ktmp[:], in0=knat[:, cs], in1=knat[:, cs]
                        )
                        nc.vector.tensor_reduce(
                            ksq[:],
                            ktmp[:].rearrange("p a b c -> p (a b) c"),
                            axis=mybir.AxisListType.X, op=mybir.AluOpType.add,
                        )
                        nc.scalar.activation(w[:], ksq[:], EXP, scale=-0.125)
                        nc.vector.tensor_mul(
                            out=vaug_[:, :, :D],
                            in0=vnat[:, cs].rearrange("p a b c -> p (a b) c"),
                            in1=w[:, :, None].to_broadcast((P, 4, D)),
                        )
                        nc.vector.tensor_copy(
                            out=vaug_[:, :, D : D + 1], in_=w[:, :, None]
                        )

                    return run

                yield allocs
                if h == 0:
                    # whole-head loads (one DMA per tensor): fewer 650ns
                    # HWDGE slots on SP's in-order queue before the first
                    # transpose can issue
                    yield loads(0, CH)
                    for pi in range(CH // 2):
                        yield piece(pi)
                    for pi in range(CH // 2):
                        yield waug(pi)
                else:
                    for pi in range(CH // 2):
                        yield piece(pi)
                    for pi in range(CH // 2):
                        yield waug(pi)

            def job_chunks(h, mb):
                """Chunks of one (head, query-block) job.

                QK + exp stream per 256-key chunk; the PV accumulations run
                chain-major at the end of the job (one query tile's full
                accumulation at a time) because interleaving accumulation
                chains within one PSUM bank corrupts them, and this also keeps
                PE's in-order SEQ from stalling on exp-dependent PV matmuls
                between QK chunks.
                """
                kts, qts, vaugs = heads[h]["kt"], heads[h]["qt"], heads[h]["vaug"]
                qt = qts[mb]  # piece mb holds exactly this block's 4 Q^T tiles
                jst = {"pgs": []}

                def sub_chunk(c):
                    def run():
                        stg = stg_pool.tile([P, 2, QB], F32, tag="stg", name="stg")
                        for r in range(2):
                            nc.tensor.matmul(
                                stg[:, r, :],
                                kts[c // 2][64 * r : 64 * r + 64, c % 2, :],
                                qt[64 * r : 64 * r + 64, :, :],
                                start=True, stop=True, skip_group_check=True,
                            )
                        pg = pg_pool.tile([P, 2, QB], BF16, tag="pg")
                        nc.scalar.activation(pg[:], stg[:], EXP, scale=0.25)
                        jst["pgs"].append((pg, c, False))

                    return run

                def diag0():
                    # chunk 2mb: keys [512mb, 512mb+256) vs all 512 queries;
                    # mask applies on query cols 0:256
                    def run():
                        c = 2 * mb
                        stg = stg_pool.tile([P, 2, QB], F32, tag="stg", name="stg")
                        for r in range(2):
                            nc.tensor.matmul(
                                stg[:, r, :],
                                kts[c // 2][64 * r : 64 * r + 64, c % 2, :],
                                qt[64 * r : 64 * r + 64, :, :],
                                start=True, stop=True, skip_group_check=True,
                            )
                        pg = pg_pool.tile([P, 2, QB], BF16, tag="pg")
                        nc.scalar.activation(pg[:], stg[:], EXP, scale=0.25)
                        nc.vector.tensor_mul(
                            out=pg[:, :, 0:256], in0=pg[:, :, 0:256], in1=M[:]
                        )
                        jst["pgs"].append((pg, c, False))

                    return run

                def diag1():
                    # chunk 2mb+1: keys [512mb+256, 512mb+512) vs query cols
                    # 256:512 only (cols 0:256 fully masked, skipped)
                    def run():
                        c = 2 * mb + 1
                        stg = stg_pool.tile([P, 2, QB], F32, tag="stg", name="stg")
                        for r in range(2):
                            nc.tensor.matmul(
                                stg[:, r, 256:],
                                kts[c // 2][64 * r : 64 * r + 64, c % 2, :],
                                qt[64 * r : 64 * r + 64, 2:4, :],
                                start=True, stop=True, skip_group_check=True,
                            )
                        pg = pg_pool.tile([P, 2, QB], BF16, tag="pg")
                        nc.scalar.activation(
                            pg[:, :, 256:], stg[:, :, 256:], EXP, scale=0.25
                        )
                        nc.vector.tensor_mul(
                            out=pg[:, :, 256:], in0=pg[:, :, 256:], in1=M[:]
                        )
                        jst["pgs"].append((pg, c, True))

                    return run

                def pv_epilogue():
                    ob = ob_pool.tile([P, 4, D + 1], F32, tag="ob", name="ob")
                    for i in range(4):
                        rel = [e for e in jst["pgs"] if not (e[2] and i < 2)]
                        for gi, (pg, c, _) in enumerate(rel):
                            for r in range(2):
                                ti = 2 * c + r
                                nc.tensor.matmul(
                                    ob[:, i, :],
                                    pg[:, r, P * i : P * i + P],
                                    vaugs[ti // 4][:, ti % 4, :],
                                    start=(gi == 0 and r == 0),
                                    stop=(gi == len(rel) - 1 and r == 1),
                                    skip_group_check=True,
                                )
                    linv = epi_pool.tile([P, 4], F32, tag="linv")
                    nc.vector.reciprocal(linv[:], ob[:, :, D])
                    o_sb = epi_pool.tile([P, 2, 2, D], F32, tag="o_sb")
                    nc.vector.tensor_mul(
                        out=o_sb[:].rearrange("p a b c -> p (a b) c"),
                        in0=ob[:, :, :D],
                        in1=linv[:, :, None].to_broadcast((P, 4, D)),
                    )
                    nc.sync.dma_start(
                        out[h, mb * QB : (mb + 1) * QB, :].rearrange(
                            "(t p r) d -> p t r d", p=P, r=2
                        ),
                        o_sb[:],
                    )

                chunks = [sub_chunk(c) for c in range(2 * mb)]
                chunks += [diag0(), diag1()]
                return chunks, pv_epilogue

            # ---- software-pipelined emission ----
            # Jobs interleave 2-deep; each job's PV+epilogue is deferred by
            # one job so it never waits at the head of PE's in-order queue
            # (its pg tiles are ready well before it's emitted).
            setup0 = list(setup_chunks(0))
            for c in setup0[: -CH // 2]:
                c()
            pending0 = setup0[-CH // 2 :]
            # prefetch the remaining heads' inputs now: SP is otherwise idle
            # until the first stores, and the transposes for these heads are
            # dripped much later
            for h_ in range(1, HPC):
                for c0 in range(0, CH, 4):
                    cs = slice(c0, c0 + 4)
                    nc.sync.dma_start(
                        knats[h_][:, cs],
                        k[h_].rearrange("(t p r) d -> p t r d", p=P, r=2)[:, cs],
                    )
                    nc.sync.dma_start(
                        qnats[h_][:, cs],
                        q[h_].rearrange("(t p r) d -> p t r d", p=P, r=2)[:, cs],
                    )
                    nc.sync.dma_start(
                        vnats[h_][:, cs],
                        v[h_].rearrange("(t p r) d -> p t r d", p=P, r=2)[:, cs],
                    )
            pending = pending0
            jobs = [(h, mb) for h in range(HPC) for mb in range(MBS)]
            active = []
            deferred = []
            ji = 0
            drip = 0
            while active or ji < len(jobs):
                while len(active) < 2 and ji < len(jobs):
                    h, mb = jobs[ji]
                    if mb == 0 and h > 0 and pending:
                        for c in pending:
                            c()
                        pending = []
                    if mb == 0 and h + 1 < HPC:
                        pending = pending + list(setup_chunks(h + 1))
                    while len(deferred) >= 2:
                        deferred.pop(0)()
                    chunks, pv_fn = job_chunks(h, mb)
                    active.append((chunks, pv_fn))
                    ji += 1
                if ji >= len(jobs) and deferred:
                    deferred.pop(0)()
                for entry in list(active):
                    chunks, pv_fn = entry
                    chunks.pop(0)()
                    drip += 1
                    if drip % 3 == 0 and pending:
                        pending.pop(0)()
                    if not chunks:
                        deferred.append(pv_fn)
                        active.remove(entry)
            for c in pending:
                c()
            for fn in deferred:
                fn()

    nc.compile()
    return nc


_NC = None


def _get_nc():
    global _NC
    if _NC is None:
        _NC = build_nc()
    return _NC


def kernel(q: np.ndarray, k: np.ndarray, v: np.ndarray) -> np.ndarray:
    from concourse.bass_utils import run_bass_kernel_spmd

    nc = _get_nc()
    qf = np.ascontiguousarray(np.asarray(q, dtype=np.float32).reshape(B * H, N, D))
    kf = np.ascontiguousarray(np.asarray(k, dtype=np.float32).reshape(B * H, N, D))
    vf = np.ascontiguousarray(np.asarray(v, dtype=np.float32).reshape(B * H, N, D))
    in_maps = [
        {
            "q": np.ascontiguousarray(qf[c * HPC : (c + 1) * HPC]),
            "k": np.ascontiguousarray(kf[c * HPC : (c + 1) * HPC]),
            "v": np.ascontiguousarray(vf[c * HPC : (c + 1) * HPC]),
        }
        for c in range(NCORES)
    ]
    res = run_bass_kernel_spmd(nc, in_maps, core_ids=list(range(NCORES)))
    outs = [res.results[c]["out"] for c in range(NCORES)]
    return np.concatenate(outs, axis=0).reshape(B, H, N, D)


if __name__ == "__main__":
    rng = np.random.default_rng(0)
    qq = rng.standard_normal((B, H, N, D), dtype=np.float32)
    kk = rng.standard_normal((B, H, N, D), dtype=np.float32)
    vv = rng.standard_normal((B, H, N, D), dtype=np.float32)
    o = kernel(q=qq, k=kk, v=vv)
    print("kernel ran, out shape", o.shape, "finite:", np.isfinite(o).all())
